# revision 20
# baseline (speedup 1.0000x reference)
"""ESPnet-style attention decoder (nn_Decoder) on 8 Trainium2 NeuronCores.

v2 strategy (8-way SPMD, one chip):
- Recurrence is 8-way tensor-parallel over the 4096 LSTM gate dim (512
  gates/core, grouped as 128 of each of i/f/o/g via a host-side row
  permutation), batch-parallel attention (4 sequences/core).
- Two AllGathers per decode step (down from three): the z1 slice of the
  previous step rides along with the att_c AllGather; z0 has its own.
- AG payloads are flat-packed f32 rows; unpack DMAs write straight into
  rotated SBUF tiles (no intermediate copies).
- e / att_c use a PSUM partition-offset diagonal trick: matmul sj writes
  partitions 4sj..4sj+3, valid row at partition 5sj, read via a
  partition-stride-5 view (no extract DMAs, no [4,2048] copies).
- softmax skips max-subtraction (|2e| << fp32 overflow); the 1/sum
  normalization is folded into the diag-copy of att_c.
- X0 = ey @ W_ih0[:, :1024]^T + biases precomputed in 128-row chunks
  (bf16), chunks 2+ emitted inside the loop so the recurrence starts
  immediately.
- Final phase: logits are output-dim-parallel (1250 vocab cols/core),
  reading z1 straight from the per-step AG output buffers; per-row
  (max, sumexp, label-logit) partials merged on the host.
"""
import os
import sys

sys.path.insert(0, "/opt/trn_rl_repo")

import numpy as np
import ml_dtypes

import concourse.bass as bass
import concourse.tile as tile
from concourse import bacc, mybir
from concourse import bass_utils

f32 = mybir.dt.float32
f32r = mybir.dt.float32r
bf16 = mybir.dt.bfloat16
FT = mybir.ActivationFunctionType
OP = mybir.AluOpType
AX = mybir.AxisListType

NC = 8
B, T, EPROJS = 32, 512, 512
DUNITS, ODIM, ATT_DIM = 1024, 10000, 320
L = 128
S = int(os.environ.get("DEC_STEPS", L + 1))   # decode steps (129)
SOS = EOS = ODIM - 1
BL = B // NC          # sequences per core (4)
GS = 4 * DUNITS // NC  # gate slice per core (512)
ZS = DUNITS // NC     # hidden slice per core (128)
OS = ODIM // NC       # vocab slice per core (1250)
ACH = [(0, 128), (128, 128), (256, 64)]  # ATT_DIM k-chunks

_BUILD_CACHE = {}


def _sap(ap, start, step, count):
    """Partition-strided view: partitions start, start+step, ... of an AP."""
    a = ap[start: start + (count - 1) * step + 1]
    return bass.AP(tensor=a.tensor, offset=a.offset,
                   ap=[[step, count]] + [list(x) for x in a.ap[1:]])


def build(steps):
    nrow = steps * B
    nch = (nrow + 127) // 128
    na = min(steps, 8)            # steps covered by x0a

    nc = bacc.Bacc("TRN2", target_bir_lowering=False, debug=False,
                   num_devices=NC)

    def din(name, shape, dt):
        return nc.dram_tensor(name, shape, dt, kind="ExternalInput")

    hs_nat = din("hs_nat", (128, BL, 4, EPROJS), f32r)
    hsT = din("hsT", (128, 4, BL * T), f32r)
    eysT = din("eysT", (128, 8, nrow), bf16)
    wih0pT = din("wih0pT", (128, 8, GS), bf16)
    x0bias = din("x0bias", (1, GS), f32)
    wencT = din("wencT", (128, 4, ATT_DIM), f32r)
    bencp = din("bencp", (128, 3), f32)
    wdecT = din("wdecT", (128, 8, ATT_DIM), f32r)
    wattT = din("wattT", (128, 4, GS), f32r)
    whh0T = din("whh0T", (128, 8, GS), f32r)
    wih1T = din("wih1T", (128, 8, GS), f32r)
    whh1T = din("whh1T", (128, 8, GS), f32r)
    bias1 = din("bias1", (1, GS), f32)
    maskb = din("maskb", (BL, T), f32)
    sel = din("sel", (B, BL), f32r)
    woutT = din("woutT", (128, 8, OS), f32r)
    boutsl = din("boutsl", (1, OS), f32)
    labels = din("labels", (128, nch), f32)
    ident = din("ident", (128, 128), f32r)
    zinit = din("zinit", (128, 8, B), f32r)

    out_stats = nc.dram_tensor("out_stats", (128, nch, 3), f32,
                               kind="ExternalOutput")

    rg = [list(range(NC))]
    ACOLS = 128 * 16            # att_c region in the combined AG (2048)
    CMBW = ACOLS + 128 * B      # + z1 region (4096) = 6144

    with tile.TileContext(nc) as tc:
        with (
            tc.tile_pool(name="dram", bufs=1, space="DRAM") as DR,
            tc.tile_pool(name="sha", bufs=steps + 1, space="DRAM") as SHA,
        ):
            x0a_dram = DR.tile([na, B, GS], f32, tag="x0a")
            if steps > na:
                x0b_dram = DR.tile([steps - na, B, GS], f32, tag="x0b")
            else:
                x0b_dram = None
            shAs = []

            with tc.tile_pool(name="persist", bufs=1) as P:
                # ------------- persistent SBUF -------------
                hs_sb = P.tile([128, BL, 4, EPROJS], f32r)
                nc.sync.dma_start(hs_sb[:], hs_nat[:])
                wdecT_sb = P.tile([128, 8, ATT_DIM], f32r)
                nc.sync.dma_start(wdecT_sb[:], wdecT[:])
                wattT_sb = P.tile([128, 4, GS], f32r)
                nc.sync.dma_start(wattT_sb[:], wattT[:])
                whh0T_sb = P.tile([128, 8, GS], f32r)
                nc.sync.dma_start(whh0T_sb[:], whh0T[:])
                wih1T_sb = P.tile([128, 8, GS], f32r)
                nc.sync.dma_start(wih1T_sb[:], wih1T[:])
                whh1T_sb = P.tile([128, 8, GS], f32r)
                nc.sync.dma_start(whh1T_sb[:], whh1T[:])
                wih0pT_sb = P.tile([128, 8, GS], bf16)
                nc.sync.dma_start(wih0pT_sb[:], wih0pT[:])
                x0bias_sb = P.tile([128, GS], f32)
                nc.sync.dma_start(
                    x0bias_sb[:],
                    bass.AP(tensor=x0bias.ap().tensor, offset=0,
                            ap=[[0, 128], [1, GS]]))
                bias1_sb = P.tile([B, GS], f32)
                nc.sync.dma_start(
                    bias1_sb[:],
                    bass.AP(tensor=bias1.ap().tensor, offset=0,
                            ap=[[0, B], [1, GS]]))
                maskb_sb = P.tile([BL, T], f32)
                nc.sync.dma_start(maskb_sb[:], maskb[:])
                sel_sb = P.tile([B, BL], f32r)
                nc.sync.dma_start(sel_sb[:], sel[:])
                ident_sb = P.tile([128, 128], f32r)
                nc.sync.dma_start(ident_sb[:], ident[:])
                pre_encT_sb = P.tile([128, 3, BL * T], f32r)
                c0_sb = P.tile([B, ZS], f32)
                nc.vector.memset(c0_sb[:], 0.0)
                c1_sb = P.tile([B, ZS], f32)
                nc.vector.memset(c1_sb[:], 0.0)

                # ------------- prologue A: pre_enc -------------
                with (
                    tc.tile_pool(name="prA", bufs=1) as PA,
                    tc.tile_pool(name="prAps", bufs=1, space="PSUM") as PAP,
                ):
                    hsT_sb = PA.tile([128, 4, BL * T], f32r, tag="hsT")
                    nc.sync.dma_start(hsT_sb[:], hsT[:])
                    wencT_sb = PA.tile([128, 4, ATT_DIM], f32r, tag="wenc")
                    nc.sync.dma_start(wencT_sb[:], wencT[:])
                    bencp_sb = PA.tile([128, 3], f32, tag="benc")
                    nc.sync.dma_start(bencp_sb[:], bencp[:])
                    for ac, (a0, aw) in enumerate(ACH):
                        ps = PAP.tile([128, BL * T], f32, tag="pe")
                        for dk in range(4):
                            for ns in range(4):
                                nc.tensor.matmul(
                                    ps[:aw, ns * 512:(ns + 1) * 512],
                                    wencT_sb[:, dk, a0:a0 + aw],
                                    hsT_sb[:, dk, ns * 512:(ns + 1) * 512],
                                    start=(dk == 0), stop=(dk == 3))
                        nc.scalar.activation(
                            out=pre_encT_sb[:aw, ac, :], in_=ps[:aw, :],
                            func=FT.Tanh, bias=bencp_sb[:aw, ac:ac + 1],
                            scale=1.0)

                # ------------- recurrence (+ inline X0 chunks) -------------
                with (
                    tc.tile_pool(name="work", bufs=2) as W,
                    tc.tile_pool(name="zrot", bufs=2) as ZR,
                    tc.tile_pool(name="eych", bufs=2) as EY,
                    tc.tile_pool(name="x0pre", bufs=2) as X0P,
                    tc.tile_pool(name="ps_a", bufs=1, space="PSUM") as PSa,
                    tc.tile_pool(name="ps_e", bufs=1, space="PSUM") as PSe,
                    tc.tile_pool(name="ps_ac", bufs=1, space="PSUM") as PSac,
                    tc.tile_pool(name="ps_g", bufs=1, space="PSUM") as PSg,
                    tc.tile_pool(name="ps_z", bufs=1, space="PSUM") as PSz,
                    tc.tile_pool(name="ps_x", bufs=1, space="PSUM") as PSx,
                    tc.tile_pool(name="bnc", bufs=2, space="DRAM") as BN,
                    tc.tile_pool(name="shb", bufs=2, space="DRAM") as SHB,
                ):
                    def emit_x0_chunk(ch):
                        cw = min(128, nrow - ch * 128)
                        ey_t = EY.tile([128, 8, 128], bf16, tag="eych")
                        nc.sync.dma_start(
                            ey_t[:, :, :cw],
                            eysT[:, :, ch * 128: ch * 128 + cw])
                        ps = PSx.tile([128, GS], f32, tag="x0")
                        for kt in range(8):
                            nc.tensor.matmul(
                                ps[:cw, :], ey_t[:, kt, :cw],
                                wih0pT_sb[:, kt, :],
                                start=(kt == 0), stop=(kt == 7))
                        g = W.tile([128, GS], f32, tag="x0g")
                        nc.vector.tensor_tensor(
                            out=g[:cw, :], in0=ps[:cw, :],
                            in1=x0bias_sb[:cw, :], op=OP.add)
                        if ch < 2:
                            dst = x0a_dram[:].rearrange("t b g -> (t b) g")
                            r0 = ch * 128
                        else:
                            dst = x0b_dram[:].rearrange("t b g -> (t b) g")
                            r0 = (ch - 2) * 128
                        nc.sync.dma_start(dst[r0: r0 + cw, :], g[:cw, :])

                    for ch in range(min(nch, 2)):
                        emit_x0_chunk(ch)

                    z0T_cur = ZR.tile([128, 8, B], f32r, tag="z0T")
                    nc.sync.dma_start(z0T_cur[:], zinit[:])
                    z1T_cur = None
                    z1sl_prev = None
                    # block-diag lhsT tiles: zeros everywhere; per-step the
                    # diagonal blocks are refreshed, so e / att_c come out
                    # as dense [4, T]/[4, EPROJS] rows with no diag-unpack
                    dbd = ZR.tile([128, 3 * BL, BL], f32r, tag="dbd")
                    nc.vector.memset(dbd[:].bitcast(f32), 0.0)
                    wbd = ZR.tile([128, 4 * BL, BL], f32r, tag="wbd")
                    nc.vector.memset(wbd[:].bitcast(f32), 0.0)

                    for t in range(steps):
                        if 2 <= t + 2 < nch:
                            emit_x0_chunk(t + 2)

                        x0_t = X0P.tile([B, GS], f32, tag="x0t")
                        if t < na:
                            nc.sync.dma_start(x0_t[:], x0a_dram[t])
                        else:
                            nc.sync.dma_start(x0_t[:], x0b_dram[t - na])

                        # dec = tanh(z0 @ WdecT), all 32 seqs
                        dec_ps = PSa.tile([B, ATT_DIM], f32, tag="a")
                        for kt in range(8):
                            nc.tensor.matmul(dec_ps[:], z0T_cur[:, kt, :],
                                             wdecT_sb[:, kt, :],
                                             start=(kt == 0), stop=(kt == 7))
                        dec_sb = W.tile([B, ATT_DIM], f32r, tag="dec")
                        nc.scalar.activation(out=dec_sb[:], in_=dec_ps[:],
                                             func=FT.Tanh)

                        # decT compact [128, 3, 4] via selector matmul
                        dT_ps = PSa.tile([128, 3, BL], f32, tag="a")
                        for ac, (a0, aw) in enumerate(ACH):
                            nc.tensor.matmul(
                                dT_ps[:aw, ac, :],
                                dec_sb[:, a0:a0 + aw],
                                sel_sb[:], start=True, stop=True)
                        # refresh the diagonal blocks of dbd
                        for sj in range(BL):
                            nc.vector.tensor_copy(
                                out=dbd[:, 3 * sj: 3 * sj + 3, sj:sj + 1],
                                in_=dT_ps[:, :, sj:sj + 1])

                        # e dense [4, T] via block-diag lhsT
                        e_ps = PSe.tile([BL, T], f32, tag="e")
                        for sj in range(BL):
                            for ac, (a0, aw) in enumerate(ACH):
                                nc.tensor.matmul(
                                    e_ps[:],
                                    dbd[:aw, 3 * sj + ac, :],
                                    pre_encT_sb[:aw, ac, sj * T:(sj + 1) * T],
                                    start=(sj == 0 and ac == 0),
                                    stop=(sj == BL - 1 and ac == 2))
                        # softmax over T (no max-sub; |2e| small):
                        # p = exp(2*e + maskb), ssum = row sums
                        e_b = W.tile([BL, T], f32, tag="eb")
                        nc.vector.scalar_tensor_tensor(
                            out=e_b[:], in0=e_ps[:],
                            scalar=2.0, in1=maskb_sb[:],
                            op0=OP.mult, op1=OP.add)
                        p_t = W.tile([BL, T], f32r, tag="pt")
                        ssum = W.tile([BL, 1], f32, tag="ssum")
                        nc.scalar.activation(
                            out=p_t[:], in_=e_b[:], func=FT.Exp,
                            accum_out=ssum[:])
                        rsum = W.tile([BL, 1], f32, tag="rsum")
                        nc.vector.reciprocal(out=rsum[:], in_=ssum[:])

                        # pT (4 transposes [4,128] -> [128,4]) -> wbd diag
                        wT_ps = PSa.tile([128, 4, BL], f32r, tag="a")
                        for tk in range(4):
                            nc.tensor.transpose(
                                wT_ps[:, tk, :],
                                p_t[:, tk * 128:(tk + 1) * 128],
                                ident_sb[0:BL, 0:BL])
                        for sj in range(BL):
                            nc.vector.tensor_copy(
                                out=wbd[:, 4 * sj: 4 * sj + 4, sj:sj + 1],
                                in_=wT_ps[:, :, sj:sj + 1])

                        # att_c dense [4, EPROJS] (unnormalized)
                        ac_ps = PSac.tile([BL, EPROJS], f32, tag="ac")
                        for sj in range(BL):
                            for tk in range(4):
                                nc.tensor.matmul(
                                    ac_ps[:],
                                    wbd[:, 4 * sj + tk, :],
                                    hs_sb[:, sj, tk, :],
                                    start=(sj == 0 and tk == 0),
                                    stop=(sj == BL - 1 and tk == 3))
                        # * (1/sum) -> own att_c [4, 512]
                        acown = W.tile([BL, EPROJS], f32, tag="acown")
                        nc.vector.tensor_scalar_mul(
                            out=acown[:], in0=ac_ps[:], scalar1=rsum[:])

                        # combined AG: own att_c rows + prev z1^T slice
                        cmbA = BN.tile([1, CMBW], f32, tag="cmbA")
                        nc.sync.dma_start(
                            cmbA[:, 0:ACOLS]
                            .rearrange("o (j d) -> (o j) d", j=BL),
                            acown[:])
                        zdst = cmbA[:, ACOLS:CMBW].rearrange(
                            "o (k b) -> (o k) b", k=128)
                        if t == 0:
                            nc.sync.dma_start(zdst,
                                              zinit[:, 0, :].bitcast(f32))
                        else:
                            nc.sync.dma_start(zdst, z1sl_prev[:])
                        shA = SHA.tile([NC, CMBW], f32, tag="shA",
                                       addr_space="Shared")
                        shAs.append(shA)
                        nc.gpsimd.collective_compute(
                            "AllGather", OP.bypass, replica_groups=rg,
                            ins=[cmbA[:]], outs=[shA[:]])
                        attall_sb = W.tile([B, EPROJS], f32r, tag="attall")
                        for r in range(NC):
                            nc.sync.dma_start(
                                attall_sb[BL * r: BL * (r + 1), :],
                                shA[r: r + 1, 0:ACOLS].rearrange(
                                    "o (j d) -> (o j) d", j=BL)
                                .bitcast(f32r))
                        z1T_cur = ZR.tile([128, 8, B], f32r, tag="z1T")
                        nc.sync.dma_start(
                            z1T_cur[:],
                            shA[:, ACOLS:CMBW].rearrange(
                                "r (k b) -> k r b", k=128).bitcast(f32r))

                        # attT (4 transposes [32,128] -> [128,32])
                        aT_ps = PSa.tile([128, 4, B], f32r, tag="a")
                        for dk in range(4):
                            nc.tensor.transpose(
                                aT_ps[:, dk, :],
                                attall_sb[:, dk * 128:(dk + 1) * 128],
                                ident_sb[0:B, 0:B])
                        attT_sb = W.tile([128, 4, B], f32r, tag="attT")
                        nc.vector.tensor_copy(out=attT_sb[:], in_=aT_ps[:])

                        # g0 = att_c @ WattT + z0 @ Whh0T (+ X0[t])
                        g0_ps = PSg.tile([B, GS], f32, tag="g0")
                        for dk in range(4):
                            nc.tensor.matmul(
                                g0_ps[:],
                                attT_sb[:, dk, :],
                                wattT_sb[:, dk, :],
                                start=(dk == 0), stop=False)
                        for kt in range(8):
                            nc.tensor.matmul(g0_ps[:], z0T_cur[:, kt, :],
                                             whh0T_sb[:, kt, :],
                                             start=False, stop=(kt == 7))
                        g0_sb = W.tile([B, GS], f32, tag="g0")
                        nc.vector.tensor_tensor(out=g0_sb[:], in0=g0_ps[:],
                                                in1=x0_t[:], op=OP.add)

                        # cell 0 (gate order i|f|o|g)
                        sifo = W.tile([B, 384], f32, tag="c_sifo")
                        nc.scalar.activation(out=sifo[:], in_=g0_sb[:, 0:384],
                                             func=FT.Sigmoid)
                        tg = W.tile([B, ZS], f32, tag="c_tg")
                        nc.scalar.activation(out=tg[:], in_=g0_sb[:, 384:512],
                                             func=FT.Tanh)
                        t1 = W.tile([B, ZS], f32, tag="c_t1")
                        nc.vector.tensor_mul(out=t1[:], in0=sifo[:, 128:256],
                                             in1=c0_sb[:])
                        t2 = W.tile([B, ZS], f32, tag="c_t2")
                        nc.vector.tensor_mul(out=t2[:], in0=sifo[:, 0:128],
                                             in1=tg[:])
                        nc.vector.tensor_add(out=c0_sb[:], in0=t1[:],
                                             in1=t2[:])
                        tc_ = W.tile([B, ZS], f32, tag="c_tc")
                        nc.scalar.activation(out=tc_[:], in_=c0_sb[:],
                                             func=FT.Tanh)
                        z0n = W.tile([B, ZS], f32r, tag="c_zn")
                        nc.vector.tensor_mul(out=z0n[:], in0=sifo[:, 256:384],
                                             in1=tc_[:])

                        # z0 slice -> [128, 32] -> AG -> z0T full
                        zT_ps = PSz.tile([128, B], f32r, tag="z")
                        nc.tensor.transpose(zT_ps[:], z0n[:],
                                            ident_sb[0:B, 0:B])
                        z0sl = W.tile([128, B], f32, tag="z0sl")
                        nc.vector.tensor_copy(out=z0sl[:], in_=zT_ps[:])
                        bnB = BN.tile([1, 128 * B], f32, tag="bnB")
                        nc.sync.dma_start(
                            bnB[:].rearrange("o (k b) -> (o k) b", k=128),
                            z0sl[:])
                        shB = SHB.tile([NC, 128 * B], f32, tag="shB",
                                       addr_space="Shared")
                        nc.gpsimd.collective_compute(
                            "AllGather", OP.bypass, replica_groups=rg,
                            ins=[bnB[:]], outs=[shB[:]])
                        z0T_cur = ZR.tile([128, 8, B], f32r, tag="z0T")
                        nc.sync.dma_start(
                            z0T_cur[:],
                            shB[:].rearrange("r (k b) -> k r b", k=128)
                            .bitcast(f32r))

                        # LSTM1 (fresh z0T, prev z1T from the ride-along)
                        g1_ps = PSg.tile([B, GS], f32, tag="g1")
                        for kt in range(8):
                            nc.tensor.matmul(g1_ps[:], z0T_cur[:, kt, :],
                                             wih1T_sb[:, kt, :],
                                             start=(kt == 0), stop=False)
                        for kt in range(8):
                            nc.tensor.matmul(g1_ps[:], z1T_cur[:, kt, :],
                                             whh1T_sb[:, kt, :],
                                             start=False, stop=(kt == 7))
                        g1_sb = W.tile([B, GS], f32, tag="g1")
                        nc.vector.tensor_tensor(
                            out=g1_sb[:], in0=g1_ps[:],
                            in1=bias1_sb[:], op=OP.add)

                        sifo1 = W.tile([B, 384], f32, tag="d_sifo")
                        nc.scalar.activation(out=sifo1[:],
                                             in_=g1_sb[:, 0:384],
                                             func=FT.Sigmoid)
                        tg1 = W.tile([B, ZS], f32, tag="d_tg")
                        nc.scalar.activation(out=tg1[:],
                                             in_=g1_sb[:, 384:512],
                                             func=FT.Tanh)
                        t11 = W.tile([B, ZS], f32, tag="d_t1")
                        nc.vector.tensor_mul(out=t11[:], in0=sifo1[:, 128:256],
                                             in1=c1_sb[:])
                        t21 = W.tile([B, ZS], f32, tag="d_t2")
                        nc.vector.tensor_mul(out=t21[:], in0=sifo1[:, 0:128],
                                             in1=tg1[:])
                        nc.vector.tensor_add(out=c1_sb[:], in0=t11[:],
                                             in1=t21[:])
                        tc1 = W.tile([B, ZS], f32, tag="d_tc")
                        nc.scalar.activation(out=tc1[:], in_=c1_sb[:],
                                             func=FT.Tanh)
                        z1n = W.tile([B, ZS], f32r, tag="d_zn")
                        nc.vector.tensor_mul(out=z1n[:], in0=sifo1[:, 256:384],
                                             in1=tc1[:])

                        z1T_ps = PSz.tile([128, B], f32r, tag="z")
                        nc.tensor.transpose(z1T_ps[:], z1n[:],
                                            ident_sb[0:B, 0:B])
                        z1sl_prev = ZR.tile([128, B], f32, tag="z1sl")
                        nc.vector.tensor_copy(out=z1sl_prev[:], in_=z1T_ps[:])

                    # final AG: last step's z1 slice
                    cmbF = BN.tile([1, 128 * B], f32, tag="cmbF")
                    nc.sync.dma_start(
                        cmbF[:].rearrange("o (k b) -> (o k) b", k=128),
                        z1sl_prev[:])
                    shF = SHA.tile([NC, 128 * B], f32, tag="shF",
                                   addr_space="Shared")
                    nc.gpsimd.collective_compute(
                        "AllGather", OP.bypass, replica_groups=rg,
                        ins=[cmbF[:]], outs=[shF[:]])

            # ------------- logits + partial log-softmax -------------
            with (
                tc.tile_pool(name="lg", bufs=2) as LG,
                tc.tile_pool(name="lg1", bufs=1) as LG1,
                tc.tile_pool(name="lgps", bufs=2, space="PSUM") as LPS,
            ):
                woutT_sb = LG1.tile([128, 8, OS], f32r, tag="wout")
                nc.sync.dma_start(woutT_sb[:], woutT[:])
                bout_sb = LG1.tile([128, OS], f32, tag="bout")
                nc.sync.dma_start(
                    bout_sb[:],
                    bass.AP(tensor=boutsl.ap().tensor, offset=0,
                            ap=[[0, 128], [1, OS]]))
                lab_sb = LG1.tile([128, nch], f32, tag="lab")
                nc.sync.dma_start(lab_sb[:], labels[:])
                iota_sb = LG1.tile([128, OS], f32, tag="iota")
                nc.gpsimd.iota(iota_sb[:], pattern=[[1, OS]], base=0,
                               channel_multiplier=0,
                               allow_small_or_imprecise_dtypes=True)
                m_all = LG1.tile([128, nch], f32, tag="m")
                s_all = LG1.tile([128, nch], f32, tag="s")
                lg_all = LG1.tile([128, nch], f32, tag="lg")

                osubs = [(0, 512), (512, 512), (1024, OS - 1024)]
                for ch in range(nch):
                    zch = LG.tile([128, 8, 4, B], f32r, tag="zch")
                    if ch == nch - 1:
                        nc.vector.memset(zch[:].bitcast(f32), 0.0)
                    for tt in range(4):
                        s = 4 * ch + tt
                        if s >= steps:
                            continue
                        if s < steps - 1:
                            src = shAs[s + 1][:, ACOLS:CMBW]
                        else:
                            src = shF[:]
                        nc.sync.dma_start(
                            zch[:, :, tt, :],
                            src.rearrange("r (k b) -> k r b", k=128)
                            .bitcast(f32r))
                    zch_f = zch[:].rearrange("k kt t b -> k kt (t b)")
                    ps = LPS.tile([128, OS], f32, tag="lps")
                    for (o0, ow) in osubs:
                        for kt in range(8):
                            nc.tensor.matmul(
                                ps[:, o0:o0 + ow], zch_f[:, kt, :],
                                woutT_sb[:, kt, o0:o0 + ow],
                                start=(kt == 0), stop=(kt == 7))
                    buf = LG.tile([128, OS], f32, tag="lbuf")
                    nc.vector.tensor_tensor(
                        out=buf[:], in0=ps[:],
                        in1=bout_sb[:], op=OP.add)
                    negm = LG.tile([128, 1], f32, tag="lnegm")
                    nc.vector.tensor_reduce(out=negm[:], in_=buf[:],
                                            op=OP.max, axis=AX.X, negate=True)
                    nc.vector.tensor_scalar_mul(
                        out=m_all[:, ch:ch + 1], in0=negm[:], scalar1=-1.0)
                    mask = LG.tile([128, OS], f32, tag="lmask")
                    nc.vector.tensor_scalar(
                        out=mask[:], in0=iota_sb[:],
                        scalar1=lab_sb[:, ch:ch + 1], scalar2=None,
                        op0=OP.is_equal)
                    prod = LG.tile([128, OS], f32, tag="lprod")
                    nc.vector.tensor_mul(out=prod[:], in0=buf[:],
                                         in1=mask[:])
                    nc.vector.tensor_reduce(
                        out=lg_all[:, ch:ch + 1], in_=prod[:],
                        op=OP.add, axis=AX.X)
                    nc.scalar.activation(
                        out=buf[:], in_=buf[:], func=FT.Exp,
                        bias=negm[:], scale=1.0,
                        accum_out=s_all[:, ch:ch + 1])

                nc.sync.dma_start(out_stats[:, :, 0], m_all[:])
                nc.sync.dma_start(out_stats[:, :, 1], s_all[:])
                nc.sync.dma_start(out_stats[:, :, 2], lg_all[:])

    nc.finalize()
    return nc


# ---------------------------------------------------------------------------
# host side
# ---------------------------------------------------------------------------

def _prep_inputs(hs_pad, hlens, ys_pad, embed_w, Wenc, benc, Wdec,
                 W_ih0, W_hh0, b_ih0, b_hh0, W_ih1, W_hh1, b_ih1, b_hh1,
                 Wout, bout, steps):
    """Shard + pack all inputs into per-core in_maps (pure data movement)."""
    f = np.float32
    hs_pad = np.asarray(hs_pad, f)
    ys_pad = np.asarray(ys_pad)
    ys_in = np.concatenate(
        [np.full((B, 1), SOS, ys_pad.dtype), ys_pad], axis=1)[:, :steps]
    ys_out = np.concatenate(
        [ys_pad, np.full((B, 1), EOS, ys_pad.dtype)], axis=1)[:, :steps]

    # gate permutation: core c's rows = 128 each of i/f/o/g
    perm = np.concatenate(
        [g * DUNITS + c * ZS + np.arange(ZS)
         for c in range(NC) for g in (0, 1, 3, 2)])

    eys = np.asarray(embed_w, f)[ys_in]                  # [B, steps, 1024]
    eysT = np.ascontiguousarray(
        eys.transpose(2, 1, 0).reshape(DUNITS, steps * B))
    eysT = np.ascontiguousarray(
        eysT.reshape(8, 128, -1).transpose(1, 0, 2)).astype(
            ml_dtypes.bfloat16)                          # [128, 8, rows]

    def kpack(M, dt=f):
        """[K, N] -> [128, K//128, N]"""
        K = M.shape[0]
        return np.ascontiguousarray(
            M.reshape(K // 128, 128, -1).transpose(1, 0, 2)).astype(dt)

    W_ih0 = np.asarray(W_ih0, f)[perm]
    W_hh0 = np.asarray(W_hh0, f)[perm]
    W_ih1 = np.asarray(W_ih1, f)[perm]
    W_hh1 = np.asarray(W_hh1, f)[perm]
    bias0 = (np.asarray(b_ih0, f) + np.asarray(b_hh0, f))[perm]
    bias1v = (np.asarray(b_ih1, f) + np.asarray(b_hh1, f))[perm]

    bencpv = np.zeros((3, 128), f)
    bencpv.reshape(-1)[:ATT_DIM] = np.asarray(benc, f)

    wencT = kpack(np.asarray(Wenc, f).T)        # [128, 4, 320]
    wdecT = kpack(np.asarray(Wdec, f).T)        # [128, 8, 320]
    identv = np.eye(128, dtype=f)
    zinitv = np.zeros((128, 8, B), f)

    Wout = np.asarray(Wout, f)
    bout_v = np.asarray(bout, f)

    ys_out_flat = ys_out.T.reshape(-1)          # row r = t*B + b
    nrow = steps * B
    nch = (nrow + 127) // 128

    in_maps = []
    for c in range(NC):
        sl = slice(GS * c, GS * (c + 1))
        seqs = slice(BL * c, BL * (c + 1))
        hs_c = hs_pad[seqs]                     # [4, 512, 512]
        hs_nat = np.ascontiguousarray(
            hs_c.reshape(BL, 4, 128, EPROJS).transpose(2, 0, 1, 3))
        hsT = np.ascontiguousarray(
            hs_c.transpose(2, 0, 1)             # [d, s, t]
            .reshape(4, 128, BL, T)
            .transpose(1, 0, 2, 3)
            .reshape(128, 4, BL * T))
        hl = np.asarray(hlens).reshape(-1)[seqs]
        maskbv = np.where(np.arange(T)[None, :] < hl[:, None],
                          0.0, -1e10).astype(f)
        selv = np.zeros((B, BL), f)
        for j in range(BL):
            selv[BL * c + j, j] = 1.0
        labv = np.full((nch * 128,), -1.0, f)
        lo = OS * c
        lb = ys_out_flat.astype(np.int64) - lo
        valid = (lb >= 0) & (lb < OS)
        labv[:nrow][valid] = lb[valid].astype(f)
        labv = labv.reshape(nch, 128).T.copy()  # [128, nch]

        in_maps.append({
            "hs_nat": hs_nat,
            "hsT": hsT,
            "eysT": eysT,
            "wih0pT": kpack(W_ih0[sl, :DUNITS].T, ml_dtypes.bfloat16),
            "x0bias": np.ascontiguousarray(bias0[sl][None]),
            "wencT": wencT,
            "bencp": np.ascontiguousarray(bencpv.T),
            "wdecT": wdecT,
            "wattT": kpack(W_ih0[sl, DUNITS:].T),
            "whh0T": kpack(W_hh0[sl].T),
            "wih1T": kpack(W_ih1[sl].T),
            "whh1T": kpack(W_hh1[sl].T),
            "bias1": np.ascontiguousarray(bias1v[sl][None]),
            "maskb": maskbv,
            "sel": selv,
            "woutT": kpack(Wout[OS * c: OS * (c + 1)].T),
            "boutsl": np.ascontiguousarray(bout_v[OS * c: OS * (c + 1)][None]),
            "labels": labv,
            "ident": identv,
            "zinit": zinitv,
        })
    return in_maps


def _combine(results, steps):
    """Merge per-core (m, S, lab) partials into (loss, acc, ppl)."""
    nrow = steps * B
    ms, ss, labs = [], [], []
    for r in results:
        st = r["out_stats"]                     # [128, nch, 3]
        ms.append(st[:, :, 0].T.reshape(-1)[:nrow])
        ss.append(st[:, :, 1].T.reshape(-1)[:nrow])
        labs.append(st[:, :, 2].T.reshape(-1)[:nrow])
    m = np.stack(ms)
    s = np.stack(ss)
    lab = np.stack(labs)
    gmax = m.max(axis=0)
    gsum = (s.astype(np.float64)
            * np.exp(m.astype(np.float64) - gmax[None])).sum(axis=0)
    lablogit = lab.sum(axis=0)
    nll = gmax.astype(np.float64) + np.log(gsum) - lablogit
    match = (lab == gmax[None]).any(axis=0)
    loss = np.float32(nll.mean() * L)
    acc = np.float32(match.mean())
    ppl = np.float32(np.exp(np.float64(loss) / B))
    return loss, acc, ppl


def kernel(**inputs):
    steps = S
    in_maps = _prep_inputs(steps=steps, **inputs)
    if steps not in _BUILD_CACHE:
        _BUILD_CACHE[steps] = build(steps)
    nc = _BUILD_CACHE[steps]
    res = bass_utils.run_bass_kernel_spmd(
        nc, in_maps, core_ids=list(range(NC)))
    return _combine(res.results, steps)
